# revision 22
# baseline (speedup 1.0000x reference)
"""Trainium2 Bass kernel for nn_AttentionHead (B=4, S=2048, DK=1024).

Single-head attention with input projections:
    qp = q @ wq.T; kp = k @ wk.T; vp = v @ wv.T
    s  = qp @ kp.T / sqrt(dk); attn = softmax(s); out = attn @ vp

Sharding: 8 cores = (batch b in 0..3) x (query-row half h in 0..1).
Each core computes the full K/V projection for its batch (duplicated
across the pair) and attention for its 1024 query rows.

Device-side layout trick: everything is kept "feature-major" so all
matmul contractions land on the partition dim with zero on-device
transposes. The host passes q/k/v/w pre-transposed; the kernel returns
out.T per core and the host transposes back.

Per core:
    kpT[e,j] = sum_d wkT[d,e] * kT[d,j]      (256 MMs)
    qpT[e,i] = sum_d wqT[d,e] * qT[d,i]      (128 MMs)
    sT[j,i]  = sum_e kpT[e,j] * qpT[e,i]     (256 MMs)
    eT[j,i]  = exp(sT/32)                     (ACT, fused scale; round-trips
                                               through DRAM to free SBUF)
    cs[i]    = sum_j eT[j,i]  via ones-matmul (broadcast over partitions)
    vp[j,e]  = sum_d vT[d,j] * wvT[d,e]      (256 MMs)
    outT[e,i]= (sum_j vp[j,e] * eT[j,i]) * (1/cs[i])   (256 MMs)

Matmuls run as float32r (fp32 bytes, single-pass PE mode, ~4x the
fp32 rate). All matmul operands are produced directly in float32r
(DMA loads and engine writes), satisfying the BIR verifier's
"rounded to FP32r" rule. Measured end-to-end relative error vs the
fp32 reference: ~4e-4.

SBUF budget is ~208KB/partition, managed as two allocation stacks
(left/right) with phase-scoped pools. Inputs stream through small
rotating chunk pools ([128,512] tiles, 2 slots per contraction tile)
in first-use order so DMA overlaps compute; 52 warm-up matmuls on a
constant tile keep the PE HAM clock at full rate while the first
input chunks land. Colsum matmuls trail their exp by one group so
the in-order PE never waits on the ACT engine.

exp(sT) round-trips through DRAM (staged exp tiles DMA out during
the score phase, streamed back in i-slice halves with a split-j
accumulation in the output phase). That frees 64KB of SBUF, which
lets wv prefetch during earlier phases via the weight-pool rotation
— the PE runs gap-free from warm-up to the last matmul and the HAM
clock stays at 2.4GHz for the whole kernel.

Measured on 8 axon-attached TRN2 cores: ~304 us HW exec time
(PE-limited; 1184 N=512 fp32r matmuls/core stream at ~233 ns each;
phases A-F all within ~2% of the matmul issue-rate floor).
"""

import numpy as np

_B, _S, _DK = 4, 2048, 1024
_HALF = _S // 2
_N_CORES = 8
_P = 128

_CACHE = {}


def _emit(tc, qT, kT, vT, wqT, wkT, wvT, outT, DK, S, HALF, mm_dt):
    import concourse.bass as bass
    from concourse import mybir

    nc = tc.nc
    ts = bass.ts
    P = _P
    NF = min(512, HALF, S, DK)
    DT = DK // P        # contraction tiles (d)
    ET = DK // P        # output-feature tiles (e)
    JT = S // P         # key tiles (j)
    ISL = HALF // NF    # query slices (i)
    JSL = S // NF       # key slices
    ESL = DK // NF      # feature slices
    JGN = S // NF       # vT chunk groups (NF//P j-tiles each)
    JPG = NF // P       # j-tiles per vT chunk
    NORM = 1.0 / float(np.sqrt(DK))
    f32 = mybir.dt.float32
    AF = mybir.ActivationFunctionType

    _cms = {}

    def opn(**kw):
        cm = tc.tile_pool(**kw)
        pool = cm.__enter__()
        _cms[id(pool)] = cm
        return pool

    def cls(*pools):
        for pool in pools:
            _cms.pop(id(pool)).__exit__(None, None, None)

    # ---------------- pools ----------------
    # LEFT stack: misc | x (stream rotation) | kpT | qpT | later vp, wv
    # RIGHT stack: stage | w (wk/wq chunks) | later eT
    misc = opn(name="misc", bufs=1, side="left")
    xp = opn(name="xp", bufs=1, side="left")
    stage = opn(name="stage", bufs=2, side="right")
    wp = opn(name="wp", bufs=1, side="right")
    psmm = opn(name="psmm", bufs=6, space="PSUM")
    psacc = opn(name="psacc", bufs=1, space="PSUM")
    dram = opn(name="dram", bufs=1, space="DRAM")
    eT_dram = dram.tile([S, HALF], mm_dt, name="et_dram")

    ones_f32 = misc.tile([P, P], f32, tag="ones_f32")
    nc.vector.memset(ones_f32[:], 1.0)
    ones = misc.tile([P, P], mm_dt, tag="ones")
    nc.vector.tensor_copy(ones[:], ones_f32[:])
    recip = misc.tile([P, HALF], f32, tag="recip")
    cs_ps = [psacc.tile([P, NF], f32, tag=f"cs{i}", name=f"cs{i}") for i in range(ISL)]

    # x-pool rotation: per-d stream chunks [P, NF], 2 slots.
    # Allocation order per d: k[0..JSL-1], q[0..ISL-1], vs[0..JGN-1].
    def x_tile(kind, d, idx):
        return xp.tile([P, NF], mm_dt, tag=f"x{d}", bufs=2, name=f"{kind}{idx}_d{d}")

    # ---------------- PE warm-up while first DMAs land ----------------
    warm_ps = psmm.tile([P, P], f32, tag="mm", name="warm_ps")
    for _ in range(52):
        nc.tensor.matmul(warm_ps[:], ones[:], ones[:], start=True, stop=True)

    # ---------------- phase A: kpT = (k @ wk.T).T ----------------
    kp_pool = opn(name="kpp", bufs=1, side="left")
    kpT = [kp_pool.tile([P, S], mm_dt, tag=f"kp{e}", name=f"kp{e}") for e in range(ET)]

    # wk/wq chunk slots [P, NF] (e-halves), 2 bufs: slot0 = wk, slot1 = wq
    EPC = NF // P  # e-tiles per w chunk
    WH = ET // EPC  # w chunks per d
    wk_c = [[None] * WH for _ in range(DT)]
    wq_c = [[None] * WH for _ in range(DT)]

    def load_w(dst, d, h, src, nm, eng=None):
        t = wp.tile([P, NF], mm_dt, tag=f"w{d}h{h}", bufs=2, name=f"{nm}{d}_{h}")
        (eng or nc.sync).dma_start(t[:], src[ts(d, P), ts(h, NF)])
        dst[d][h] = t

    def w_slice(c, d, e):
        return c[d][e // EPC][:, ts(e % EPC, P)]

    k_c = [[None] * JSL for _ in range(DT)]
    q_c = [[None] * ISL for _ in range(DT)]
    vs_c = [[None] * JGN for _ in range(DT)]

    # first-use-ordered input streaming: wk[*][h0], k[*][js0], wk[*][h1..]
    for d in range(DT):
        load_w(wk_c, d, 0, wkT, "wk")
    for d in range(DT):
        k_c[d][0] = x_tile("k", d, 0)
        nc.gpsimd.dma_start(k_c[d][0][:], kT[ts(d, P), ts(0, NF)])
    for h in range(1, WH):
        for d in range(DT):
            load_w(wk_c, d, h, wkT, "wk")

    for js in range(JSL):
        if js + 1 < JSL:  # prefetch next k slab
            for d in range(DT):
                k_c[d][js + 1] = x_tile("k", d, js + 1)
                nc.gpsimd.dma_start(k_c[d][js + 1][:], kT[ts(d, P), ts(js + 1, NF)])
        if js == min(1, JSL - 1):  # wq loads ride behind early k prefetches
            for h in range(WH):
                for d in range(DT):
                    load_w(wq_c, d, h, wqT, "wq")
        if js == min(2, JSL - 1):  # q[isl0] into freed k slots
            for d in range(DT):
                q_c[d][0] = x_tile("q", d, 0)
                nc.sync.dma_start(q_c[d][0][:], qT[ts(d, P), ts(0, NF)])
        if js == JSL - 1:  # remaining q slices
            for isl in range(1, ISL):
                for d in range(DT):
                    q_c[d][isl] = x_tile("q", d, isl)
                    nc.sync.dma_start(q_c[d][isl][:], qT[ts(d, P), ts(isl, NF)])
        for e in range(ET):
            ps = psmm.tile([P, NF], f32, tag="mm")
            for d in range(DT):
                nc.tensor.matmul(
                    ps[:],
                    w_slice(wk_c, d, e),
                    k_c[d][js][:],
                    start=(d == 0),
                    stop=(d == DT - 1),
                )
            nc.vector.tensor_copy(kpT[e][:, ts(js, NF)], ps[:])

    # ---------------- phase B: qpT = (q @ wq.T).T ----------------
    qp_pool = opn(name="qpp", bufs=1, side="left")
    qpT = [
        qp_pool.tile([P, HALF], mm_dt, tag=f"qp{e}", name=f"qp{e}") for e in range(ET)
    ]
    for isl in range(ISL):
        for e in range(ET):
            ps = psmm.tile([P, NF], f32, tag="mm")
            for d in range(DT):
                nc.tensor.matmul(
                    ps[:],
                    w_slice(wq_c, d, e),
                    q_c[d][isl][:],
                    start=(d == 0),
                    stop=(d == DT - 1),
                )
            nc.vector.tensor_copy(qpT[e][:, ts(isl, NF)], ps[:])
    # wv rides the w-rotation (slot freed when wk releases at end of A),
    # so it loads during B/C — no stall at the C->E boundary.
    wv_c = [[None] * WH for _ in range(DT)]
    for h in range(WH):
        for d in range(DT):
            load_w(wv_c, d, h, wvT, "wv")

    # ---------------- phase C: sT -> exp -> eT_dram (+ colsum), vs prefetch ----
    etsp = opn(name="etsp", bufs=1, side="right")
    # vs chunks 0/1 drain as soon as q slots free (mid/end of phase B)
    for g in range(min(2, JGN)):
        for d in range(DT):
            vs_c[d][g] = x_tile("vs", d, g)
            nc.sync.dma_start(vs_c[d][g][:], vT[ts(d, P), ts(g, NF)])
    pending_cs = []
    for j in range(JT):
        for isl in range(ISL):
            ps = psmm.tile([P, NF], f32, tag="mm")
            for e in range(ET):
                nc.tensor.matmul(
                    ps[:],
                    kpT[e][:, ts(j, P)],
                    qpT[e][:, ts(isl, NF)],
                    start=(e == 0),
                    stop=(e == ET - 1),
                )
            st = etsp.tile([P, NF], mm_dt, tag="ets", bufs=3, name=f"ets{j}_{isl}")
            nc.scalar.activation(st[:], ps[:], AF.Exp, scale=NORM)
            nc.sync.dma_start(eT_dram[ts(j, P), ts(isl, NF)], st[:])
            pending_cs.append((j, isl, st))
            if len(pending_cs) > 1:
                pj, pisl, pst = pending_cs.pop(0)
                nc.tensor.matmul(
                    cs_ps[pisl][:],
                    ones[:],
                    pst[:],
                    start=(pj == 0),
                    stop=(pj == JT - 1),
                )
    for pj, pisl, pst in pending_cs:
        nc.tensor.matmul(
            cs_ps[pisl][:],
            ones[:],
            pst[:],
            start=(pj == 0),
            stop=(pj == JT - 1),
        )
    for isl in range(ISL):
        nc.vector.reciprocal(recip[:, ts(isl, NF)], cs_ps[isl][:])
    cls(etsp)
    cls(qp_pool, kp_pool)
    cls(psacc)

    # ---------------- phase E: vp = v @ wv.T ----------------
    vp_pool = opn(name="vpp", bufs=1, side="left")
    vp = [vp_pool.tile([P, DK], mm_dt, tag=f"vp{j}", name=f"vp{j}") for j in range(JT)]
    # eT comes back from DRAM in halves during E/F (tag rotation per j%8)
    ethp = opn(name="ethp", bufs=1, side="left")
    eth = [[None] * JT for _ in range(ISL)]

    def load_eth(isl, jlist):
        for j in jlist:
            t = ethp.tile(
                [P, NF], mm_dt, tag=f"eh{j % 8}", bufs=2, name=f"eh{isl}_{j}"
            )
            nc.sync.dma_start(t[:], eT_dram[ts(j, P), ts(isl, NF)])
            eth[isl][j] = t

    load_eth(0, range(JT // 2))
    load_eth(0, range(JT // 2, JT))
    for g in range(JGN):
        if g + 2 < JGN:  # double-buffered vs prefetch
            gg = g + 2
            for d in range(DT):
                vs_c[d][gg] = x_tile("vs", d, gg)
                nc.sync.dma_start(vs_c[d][gg][:], vT[ts(d, P), ts(gg, NF)])
        for jin in range(JPG):
            j = g * JPG + jin
            for es in range(ESL):
                ps = psmm.tile([P, NF], f32, tag="mm")
                for d in range(DT):
                    nc.tensor.matmul(
                        ps[:],
                        vs_c[d][g][:, ts(jin, P)],
                        wv_c[d][es][:],
                        start=(d == 0),
                        stop=(d == DT - 1),
                    )
                nc.vector.tensor_copy(vp[j][:, ts(es, NF)], ps[:])
    cls(wp)
    cls(psmm)

    # ---------------- phase F: outT = (eT.T @ vp).T * recip ----------------
    # Two j-half passes per i-slice so eth tiles release mid-slice and the
    # next slice's eth loads prefetch without a stall. One PSUM bank per e.
    pf = opn(name="pf", bufs=1, space="PSUM")
    JH = JT // 2
    for isl in range(ISL):
        pft = [
            pf.tile([P, NF], f32, tag=f"pf{e}", name=f"pf{e}_{isl}")
            for e in range(ET)
        ]
        for e in range(ET):
            for j in range(JH):
                nc.tensor.matmul(
                    pft[e][:],
                    vp[j][:, ts(e, P)],
                    eth[isl][j][:],
                    start=(j == 0),
                    stop=False,
                )
        if isl + 1 < ISL:
            load_eth(isl + 1, range(JH))
        for e in range(ET):
            for j in range(JH, JT):
                nc.tensor.matmul(
                    pft[e][:],
                    vp[j][:, ts(e, P)],
                    eth[isl][j][:],
                    start=False,
                    stop=(j == JT - 1),
                )
            ot = stage.tile([P, NF], f32, tag="ost")
            nc.vector.tensor_mul(ot[:], pft[e][:], recip[:, ts(isl, NF)])
            nc.sync.dma_start(outT[ts(e, P), ts(isl, NF)], ot[:])
        if isl + 1 < ISL:
            load_eth(isl + 1, range(JH, JT))
    cls(ethp, vp_pool, xp, misc, stage, pf, dram)


def build_program(DK=_DK, S=_S, HALF=_HALF, mm_dtype="float32r"):
    """Build + compile the per-core Bass program. Returns the Bacc object."""
    import concourse.tile as tile
    from concourse import bacc, mybir

    f32 = mybir.dt.float32
    mm_dt = getattr(mybir.dt, mm_dtype)

    nc = bacc.Bacc(
        "TRN2",
        target_bir_lowering=False,
        debug=False,
        enable_asserts=False,
        num_devices=_N_CORES,
    )
    qT = nc.dram_tensor("qt", (DK, HALF), mm_dt, kind="ExternalInput").ap()
    kT = nc.dram_tensor("kt", (DK, S), mm_dt, kind="ExternalInput").ap()
    vT = nc.dram_tensor("vt", (DK, S), mm_dt, kind="ExternalInput").ap()
    wqT = nc.dram_tensor("wqt", (DK, DK), mm_dt, kind="ExternalInput").ap()
    wkT = nc.dram_tensor("wkt", (DK, DK), mm_dt, kind="ExternalInput").ap()
    wvT = nc.dram_tensor("wvt", (DK, DK), mm_dt, kind="ExternalInput").ap()
    outT = nc.dram_tensor("outt", (DK, HALF), f32, kind="ExternalOutput").ap()

    with tile.TileContext(nc) as tc:
        _emit(tc, qT, kT, vT, wqT, wkT, wvT, outT, DK, S, HALF, mm_dt)
    nc.compile()
    return nc


def _in_maps(q, k, v, wq, wk, wv):
    """Shard full inputs into 8 per-core input maps (host-side transposes)."""
    wqT = np.ascontiguousarray(wq.T)
    wkT = np.ascontiguousarray(wk.T)
    wvT = np.ascontiguousarray(wv.T)
    kT_b = [np.ascontiguousarray(k[b].T) for b in range(_B)]
    vT_b = [np.ascontiguousarray(v[b].T) for b in range(_B)]
    maps = []
    for c in range(_N_CORES):
        b, h = divmod(c, 2)
        qT = np.ascontiguousarray(q[b, h * _HALF : (h + 1) * _HALF, :].T)
        maps.append(
            {
                "qt": qT,
                "kt": kT_b[b],
                "vt": vT_b[b],
                "wqt": wqT,
                "wkt": wkT,
                "wvt": wvT,
            }
        )
    return maps


def kernel(q, k, v, wq, wk, wv):
    from concourse.bass_utils import run_bass_kernel_spmd

    q = np.asarray(q, np.float32)
    k = np.asarray(k, np.float32)
    v = np.asarray(v, np.float32)
    wq = np.asarray(wq, np.float32)
    wk = np.asarray(wk, np.float32)
    wv = np.asarray(wv, np.float32)

    if "nc" not in _CACHE:
        _CACHE["nc"] = build_program()
    nc = _CACHE["nc"]

    res = run_bass_kernel_spmd(
        nc, _in_maps(q, k, v, wq, wk, wv), core_ids=list(range(_N_CORES))
    )

    out = np.empty((_B, _S, _DK), np.float32)
    for c in range(_N_CORES):
        b, h = divmod(c, 2)
        out[b, h * _HALF : (h + 1) * _HALF, :] = res.results[c]["outt"].T
    return out


# revision 23
# speedup vs baseline: 1.0298x; 1.0298x over previous
"""Trainium2 Bass kernel for nn_AttentionHead (B=4, S=2048, DK=1024).

Single-head attention with input projections:
    qp = q @ wq.T; kp = k @ wk.T; vp = v @ wv.T
    s  = qp @ kp.T / sqrt(dk); attn = softmax(s); out = attn @ vp

Sharding: 8 cores = (batch b in 0..3) x (query-row half h in 0..1).
Each core computes the full K/V projection for its batch (duplicated
across the pair) and attention for its 1024 query rows.

Device-side layout trick: everything is kept "feature-major" so all
matmul contractions land on the partition dim with zero on-device
transposes. The host passes q/k/v/w pre-transposed; the kernel returns
out.T per core and the host transposes back.

Per core:
    kpT[e,j] = sum_d wkT[d,e] * kT[d,j]      (256 MMs)
    qpT[e,i] = sum_d wqT[d,e] * qT[d,i]      (128 MMs)
    sT[j,i]  = sum_e kpT[e,j] * qpT[e,i]     (256 MMs)
    eT[j,i]  = exp(sT/32)                     (ACT, fused scale; round-trips
                                               through DRAM to free SBUF)
    cs[i]    = sum_j eT[j,i]  via ones-matmul (broadcast over partitions)
    vp[j,e]  = sum_d vT[d,j] * wvT[d,e]      (256 MMs)
    outT[e,i]= (sum_j vp[j,e] * eT[j,i]) * (1/cs[i])   (256 MMs)

Matmuls run as float32r (fp32 bytes, single-pass PE mode, ~4x the
fp32 rate). All matmul operands are produced directly in float32r
(DMA loads and engine writes), satisfying the BIR verifier's
"rounded to FP32r" rule. Measured end-to-end relative error vs the
fp32 reference: ~4e-4.

SBUF budget is ~208KB/partition, managed as two allocation stacks
(left/right) with phase-scoped pools. Inputs stream through small
rotating chunk pools ([128,512] tiles, 2 slots per contraction tile)
in first-use order so DMA overlaps compute; 52 warm-up matmuls on a
constant tile keep the PE HAM clock at full rate while the first
input chunks land. Colsum matmuls trail their exp by one group so
the in-order PE never waits on the ACT engine.

exp(sT) round-trips through DRAM (staged exp tiles DMA out during
the score phase, streamed back in i-slice halves with a split-j
accumulation in the output phase). That frees 64KB of SBUF, which
lets wv prefetch during earlier phases via the weight-pool rotation
— the PE runs gap-free from warm-up to the last matmul and the HAM
clock stays at 2.4GHz for the whole kernel.

Measured on 8 axon-attached TRN2 cores: ~304 us HW exec time
(PE-limited; 1184 N=512 fp32r matmuls/core stream at ~233 ns each;
phases A-F all within ~2% of the matmul issue-rate floor).
"""

import numpy as np

_B, _S, _DK = 4, 2048, 1024
_HALF = _S // 2
_N_CORES = 8
_P = 128

_CACHE = {}


def _emit(tc, qT, kT, vT, wqT, wkT, wvT, outT, DK, S, HALF, mm_dt):
    import concourse.bass as bass
    from concourse import mybir

    nc = tc.nc
    ts = bass.ts
    P = _P
    NF = min(512, HALF, S, DK)
    DT = DK // P        # contraction tiles (d)
    ET = DK // P        # output-feature tiles (e)
    JT = S // P         # key tiles (j)
    ISL = HALF // NF    # query slices (i)
    JSL = S // NF       # key slices
    ESL = DK // NF      # feature slices
    JGN = S // NF       # vT chunk groups (NF//P j-tiles each)
    JPG = NF // P       # j-tiles per vT chunk
    NORM = 1.0 / float(np.sqrt(DK))
    f32 = mybir.dt.float32
    AF = mybir.ActivationFunctionType

    _cms = {}

    def opn(**kw):
        cm = tc.tile_pool(**kw)
        pool = cm.__enter__()
        _cms[id(pool)] = cm
        return pool

    def cls(*pools):
        for pool in pools:
            _cms.pop(id(pool)).__exit__(None, None, None)

    # ---------------- pools ----------------
    # LEFT stack: misc | x (stream rotation) | kpT | qpT | later vp, wv
    # RIGHT stack: stage | w (wk/wq chunks) | later eT
    misc = opn(name="misc", bufs=1, side="left")
    xp = opn(name="xp", bufs=1, side="left")
    stage = opn(name="stage", bufs=2, side="right")
    wp = opn(name="wp", bufs=1, side="right")
    psmm = opn(name="psmm", bufs=6, space="PSUM")
    psacc = opn(name="psacc", bufs=1, space="PSUM")
    dram = opn(name="dram", bufs=1, space="DRAM")
    eT_dram = dram.tile([S, HALF], mm_dt, name="et_dram")

    ones_f32 = misc.tile([P, P], f32, tag="ones_f32")
    nc.vector.memset(ones_f32[:], 1.0)
    ones = misc.tile([P, P], mm_dt, tag="ones")
    nc.vector.tensor_copy(ones[:], ones_f32[:])
    recip = misc.tile([P, HALF], f32, tag="recip")
    cs_ps = [psacc.tile([P, NF], f32, tag=f"cs{i}", name=f"cs{i}") for i in range(ISL)]

    # x-pool rotation: per-d stream chunks [P, NF], 2 slots.
    # Allocation order per d: k[0..JSL-1], q[0..ISL-1], vs[0..JGN-1].
    def x_tile(kind, d, idx):
        return xp.tile([P, NF], mm_dt, tag=f"x{d}", bufs=2, name=f"{kind}{idx}_d{d}")

    # ---------------- PE warm-up while first DMAs land ----------------
    warm_ps = psmm.tile([P, P], f32, tag="mm", name="warm_ps")
    for _ in range(52):
        nc.tensor.matmul(warm_ps[:], ones[:], ones[:], start=True, stop=True)

    # ---------------- phase A: kpT = (k @ wk.T).T ----------------
    kp_pool = opn(name="kpp", bufs=1, side="left")
    kpT = [kp_pool.tile([P, S], mm_dt, tag=f"kp{e}", name=f"kp{e}") for e in range(ET)]

    # wk/wq chunk slots [P, NF] (e-halves), 2 bufs: slot0 = wk, slot1 = wq
    EPC = NF // P  # e-tiles per w chunk
    WH = ET // EPC  # w chunks per d
    wk_c = [[None] * WH for _ in range(DT)]
    wq_c = [[None] * WH for _ in range(DT)]

    def load_w(dst, d, h, src, nm, eng=None):
        t = wp.tile([P, NF], mm_dt, tag=f"w{d}h{h}", bufs=2, name=f"{nm}{d}_{h}")
        (eng or nc.sync).dma_start(t[:], src[ts(d, P), ts(h, NF)])
        dst[d][h] = t

    def w_slice(c, d, e):
        return c[d][e // EPC][:, ts(e % EPC, P)]

    k_c = [[None] * JSL for _ in range(DT)]
    q_c = [[None] * ISL for _ in range(DT)]
    vs_c = [[None] * JGN for _ in range(DT)]

    # first-use-ordered input streaming: wk[*][h0], k[*][js0], wk[*][h1..]
    for d in range(DT):
        load_w(wk_c, d, 0, wkT, "wk")
    for d in range(DT):
        k_c[d][0] = x_tile("k", d, 0)
        nc.sync.dma_start(k_c[d][0][:], kT[ts(d, P), ts(0, NF)])
    for h in range(1, WH):
        for d in range(DT):
            load_w(wk_c, d, h, wkT, "wk")

    for js in range(JSL):
        if js + 1 < JSL:  # prefetch next k slab
            for d in range(DT):
                k_c[d][js + 1] = x_tile("k", d, js + 1)
                nc.sync.dma_start(k_c[d][js + 1][:], kT[ts(d, P), ts(js + 1, NF)])
        if js == min(1, JSL - 1):  # wq loads ride behind early k prefetches
            for h in range(WH):
                for d in range(DT):
                    load_w(wq_c, d, h, wqT, "wq")
        if js == min(2, JSL - 1):  # q[isl0] into freed k slots
            for d in range(DT):
                q_c[d][0] = x_tile("q", d, 0)
                nc.sync.dma_start(q_c[d][0][:], qT[ts(d, P), ts(0, NF)])
        if js == JSL - 1:  # remaining q slices
            for isl in range(1, ISL):
                for d in range(DT):
                    q_c[d][isl] = x_tile("q", d, isl)
                    nc.sync.dma_start(q_c[d][isl][:], qT[ts(d, P), ts(isl, NF)])
        for e in range(ET):
            ps = psmm.tile([P, NF], f32, tag="mm")
            for d in range(DT):
                nc.tensor.matmul(
                    ps[:],
                    w_slice(wk_c, d, e),
                    k_c[d][js][:],
                    start=(d == 0),
                    stop=(d == DT - 1),
                )
            nc.vector.tensor_copy(kpT[e][:, ts(js, NF)], ps[:])

    # ---------------- phase B: qpT = (q @ wq.T).T ----------------
    qp_pool = opn(name="qpp", bufs=1, side="left")
    qpT = [
        qp_pool.tile([P, HALF], mm_dt, tag=f"qp{e}", name=f"qp{e}") for e in range(ET)
    ]
    for isl in range(ISL):
        for e in range(ET):
            ps = psmm.tile([P, NF], f32, tag="mm")
            for d in range(DT):
                nc.tensor.matmul(
                    ps[:],
                    w_slice(wq_c, d, e),
                    q_c[d][isl][:],
                    start=(d == 0),
                    stop=(d == DT - 1),
                )
            nc.vector.tensor_copy(qpT[e][:, ts(isl, NF)], ps[:])
    # wv rides the w-rotation (slot freed when wk releases at end of A),
    # so it loads during B/C — no stall at the C->E boundary.
    wv_c = [[None] * WH for _ in range(DT)]
    for h in range(WH):
        for d in range(DT):
            load_w(wv_c, d, h, wvT, "wv")

    # ---------------- phase C: sT -> exp -> eT_dram (+ colsum), vs prefetch ----
    etsp = opn(name="etsp", bufs=1, side="right")
    # vs chunks 0/1 drain as soon as q slots free (mid/end of phase B)
    for g in range(min(2, JGN)):
        for d in range(DT):
            vs_c[d][g] = x_tile("vs", d, g)
            nc.sync.dma_start(vs_c[d][g][:], vT[ts(d, P), ts(g, NF)])
    pending_cs = []
    for j in range(JT):
        for isl in range(ISL):
            ps = psmm.tile([P, NF], f32, tag="mm")
            for e in range(ET):
                nc.tensor.matmul(
                    ps[:],
                    kpT[e][:, ts(j, P)],
                    qpT[e][:, ts(isl, NF)],
                    start=(e == 0),
                    stop=(e == ET - 1),
                )
            st = etsp.tile([P, NF], mm_dt, tag="ets", bufs=3, name=f"ets{j}_{isl}")
            nc.scalar.activation(st[:], ps[:], AF.Exp, scale=NORM)
            nc.sync.dma_start(eT_dram[ts(j, P), ts(isl, NF)], st[:])
            pending_cs.append((j, isl, st))
            if len(pending_cs) > 1:
                pj, pisl, pst = pending_cs.pop(0)
                nc.tensor.matmul(
                    cs_ps[pisl][:],
                    ones[:],
                    pst[:],
                    start=(pj == 0),
                    stop=(pj == JT - 1),
                )
    for pj, pisl, pst in pending_cs:
        nc.tensor.matmul(
            cs_ps[pisl][:],
            ones[:],
            pst[:],
            start=(pj == 0),
            stop=(pj == JT - 1),
        )
    for isl in range(ISL):
        nc.vector.reciprocal(recip[:, ts(isl, NF)], cs_ps[isl][:])
    cls(etsp)
    cls(qp_pool, kp_pool)
    cls(psacc)

    # ---------------- phase E: vp = v @ wv.T ----------------
    vp_pool = opn(name="vpp", bufs=1, side="left")
    vp = [vp_pool.tile([P, DK], mm_dt, tag=f"vp{j}", name=f"vp{j}") for j in range(JT)]
    # eT comes back from DRAM in halves during E/F (tag rotation per j%8)
    ethp = opn(name="ethp", bufs=1, side="left")
    eth = [[None] * JT for _ in range(ISL)]

    def load_eth(isl, jlist):
        for j in jlist:
            t = ethp.tile(
                [P, NF], mm_dt, tag=f"eh{j % 8}", bufs=2, name=f"eh{isl}_{j}"
            )
            nc.sync.dma_start(t[:], eT_dram[ts(j, P), ts(isl, NF)])
            eth[isl][j] = t

    load_eth(0, range(JT // 2))
    load_eth(0, range(JT // 2, JT))
    for g in range(JGN):
        if g + 2 < JGN:  # double-buffered vs prefetch
            gg = g + 2
            for d in range(DT):
                vs_c[d][gg] = x_tile("vs", d, gg)
                nc.sync.dma_start(vs_c[d][gg][:], vT[ts(d, P), ts(gg, NF)])
        for jin in range(JPG):
            j = g * JPG + jin
            for es in range(ESL):
                ps = psmm.tile([P, NF], f32, tag="mm")
                for d in range(DT):
                    nc.tensor.matmul(
                        ps[:],
                        vs_c[d][g][:, ts(jin, P)],
                        wv_c[d][es][:],
                        start=(d == 0),
                        stop=(d == DT - 1),
                    )
                nc.vector.tensor_copy(vp[j][:, ts(es, NF)], ps[:])
    cls(wp)
    cls(psmm)

    # ---------------- phase F: outT = (eT.T @ vp).T * recip ----------------
    # Two j-half passes per i-slice so eth tiles release mid-slice and the
    # next slice's eth loads prefetch without a stall. One PSUM bank per e.
    pf = opn(name="pf", bufs=1, space="PSUM")
    JH = JT // 2
    for isl in range(ISL):
        pft = [
            pf.tile([P, NF], f32, tag=f"pf{e}", name=f"pf{e}_{isl}")
            for e in range(ET)
        ]
        for e in range(ET):
            for j in range(JH):
                nc.tensor.matmul(
                    pft[e][:],
                    vp[j][:, ts(e, P)],
                    eth[isl][j][:],
                    start=(j == 0),
                    stop=False,
                )
        if isl + 1 < ISL:
            load_eth(isl + 1, range(JH))
        for e in range(ET):
            for j in range(JH, JT):
                nc.tensor.matmul(
                    pft[e][:],
                    vp[j][:, ts(e, P)],
                    eth[isl][j][:],
                    start=False,
                    stop=(j == JT - 1),
                )
            ot = stage.tile([P, NF], f32, tag="ost")
            nc.vector.tensor_mul(ot[:], pft[e][:], recip[:, ts(isl, NF)])
            nc.sync.dma_start(outT[ts(e, P), ts(isl, NF)], ot[:])
        if isl + 1 < ISL:
            load_eth(isl + 1, range(JH, JT))
    cls(ethp, vp_pool, xp, misc, stage, pf, dram)


def build_program(DK=_DK, S=_S, HALF=_HALF, mm_dtype="float32r"):
    """Build + compile the per-core Bass program. Returns the Bacc object."""
    import concourse.tile as tile
    from concourse import bacc, mybir

    f32 = mybir.dt.float32
    mm_dt = getattr(mybir.dt, mm_dtype)

    nc = bacc.Bacc(
        "TRN2",
        target_bir_lowering=False,
        debug=False,
        enable_asserts=False,
        num_devices=_N_CORES,
    )
    qT = nc.dram_tensor("qt", (DK, HALF), mm_dt, kind="ExternalInput").ap()
    kT = nc.dram_tensor("kt", (DK, S), mm_dt, kind="ExternalInput").ap()
    vT = nc.dram_tensor("vt", (DK, S), mm_dt, kind="ExternalInput").ap()
    wqT = nc.dram_tensor("wqt", (DK, DK), mm_dt, kind="ExternalInput").ap()
    wkT = nc.dram_tensor("wkt", (DK, DK), mm_dt, kind="ExternalInput").ap()
    wvT = nc.dram_tensor("wvt", (DK, DK), mm_dt, kind="ExternalInput").ap()
    outT = nc.dram_tensor("outt", (DK, HALF), f32, kind="ExternalOutput").ap()

    with tile.TileContext(nc) as tc:
        _emit(tc, qT, kT, vT, wqT, wkT, wvT, outT, DK, S, HALF, mm_dt)
    nc.compile()
    return nc


def _in_maps(q, k, v, wq, wk, wv):
    """Shard full inputs into 8 per-core input maps (host-side transposes)."""
    wqT = np.ascontiguousarray(wq.T)
    wkT = np.ascontiguousarray(wk.T)
    wvT = np.ascontiguousarray(wv.T)
    kT_b = [np.ascontiguousarray(k[b].T) for b in range(_B)]
    vT_b = [np.ascontiguousarray(v[b].T) for b in range(_B)]
    maps = []
    for c in range(_N_CORES):
        b, h = divmod(c, 2)
        qT = np.ascontiguousarray(q[b, h * _HALF : (h + 1) * _HALF, :].T)
        maps.append(
            {
                "qt": qT,
                "kt": kT_b[b],
                "vt": vT_b[b],
                "wqt": wqT,
                "wkt": wkT,
                "wvt": wvT,
            }
        )
    return maps


def kernel(q, k, v, wq, wk, wv):
    from concourse.bass_utils import run_bass_kernel_spmd

    q = np.asarray(q, np.float32)
    k = np.asarray(k, np.float32)
    v = np.asarray(v, np.float32)
    wq = np.asarray(wq, np.float32)
    wk = np.asarray(wk, np.float32)
    wv = np.asarray(wv, np.float32)

    if "nc" not in _CACHE:
        _CACHE["nc"] = build_program()
    nc = _CACHE["nc"]

    res = run_bass_kernel_spmd(
        nc, _in_maps(q, k, v, wq, wk, wv), core_ids=list(range(_N_CORES))
    )

    out = np.empty((_B, _S, _DK), np.float32)
    for c in range(_N_CORES):
        b, h = divmod(c, 2)
        out[b, h * _HALF : (h + 1) * _HALF, :] = res.results[c]["outt"].T
    return out
